# revision 12
# baseline (speedup 1.0000x reference)
"""Multi-head causal attention w/ RoPE on 8 Trainium2 NeuronCores.

Tensor-parallel over heads: each core owns 2 of 16 heads (both batches),
computes qkv projection / RoPE / attention / output projection for its
heads, and returns a partial [2, 2048, 2048] output (its heads' slice of
the residual). The host sums the 8 partials (the K-split of the output
projection), which is the unshard for this sharding.

Precision: fp32r (TF32-like, full PE rate) for qkv/scores matmuls; bf16
for probs @ v and the output projection. Softmax skips max-subtraction
(scores are bounded ~25 here; exp stays far from fp32 overflow).
"""
import os
import sys

for _p in ("/root/.axon_site", "/root/.axon_site/_ro/trn_rl_repo",
           "/root/.axon_site/_ro/pypackages", "/opt/trn_rl_repo"):
    if os.path.isdir(_p) and _p not in sys.path:
        sys.path.append(_p)

import numpy as np
import ml_dtypes

import concourse.bacc as bacc
import concourse.tile as tile
from concourse import mybir, masks
from concourse.bass_utils import run_bass_kernel_spmd

N_CORES = 8
B = 2
S = 2048
D = 2048
H = 16
DH = 128
HPC = H // N_CORES          # heads per core = 2
ROPE_BASE = 10000.0
SM_SCALE = DH ** -0.5

SC = 512                    # stage-A s-chunk: one N=512 matmul per stationary
QC = 512                    # attention q-chunk
NKT = S // 128              # 16 k-tiles per sequence
N_SC = S // SC
N_QC = S // QC

f32 = mybir.dt.float32
f32r = mybir.dt.float32r
bf16 = mybir.dt.bfloat16
Exp = mybir.ActivationFunctionType.Exp

LAST_RESULTS = None         # BassKernelResults of the most recent run
_NC_CACHE = {}


def build_nc():
    nc = bacc.Bacc(trn_type="TRN2", debug=False)

    xT = nc.dram_tensor("xT", [B, D, S], f32r, kind="ExternalInput").ap()
    w = nc.dram_tensor("wqkvT", [D, 3 * HPC * DH], f32r, kind="ExternalInput").ap()
    wo = nc.dram_tensor("woT", [HPC * DH, D], bf16, kind="ExternalInput").ap()
    cos = nc.dram_tensor("cosT", [DH, S], f32, kind="ExternalInput").ap()
    sin = nc.dram_tensor("sinT", [DH, S], f32, kind="ExternalInput").ap()
    msk = nc.dram_tensor("masks", [DH, QC + 384], bf16, kind="ExternalInput").ap()
    out = nc.dram_tensor("out", [B, S, D], f32, kind="ExternalOutput").ap()

    with tile.TileContext(nc) as tc:
        with (
            tc.tile_pool(name="const", bufs=1) as p_const,
            tc.tile_pool(name="batch", bufs=1) as p_batch,
            tc.tile_pool(name="xch", bufs=2) as p_x,
            tc.tile_pool(name="vtc", bufs=1) as p_vt,
            tc.tile_pool(name="tmp", bufs=2) as p_tmp,
            tc.tile_pool(name="probs", bufs=6) as p_probs,
            tc.tile_pool(name="rec", bufs=1) as p_rec,
            tc.tile_pool(name="ostage", bufs=2) as p_ost,
            tc.tile_pool(name="psMain", bufs=3, space="PSUM") as ps_main,
            tc.tile_pool(name="psDen", bufs=1, space="PSUM") as ps_den,
            tc.tile_pool(name="psZ", bufs=2, space="PSUM") as ps_z,
            tc.tile_pool(name="psO", bufs=2, space="PSUM") as ps_o,
        ):
            # ---- constants -------------------------------------------------
            w_sb = p_const.tile([128, NKT, 3 * HPC * DH], f32r)     # 48KB/p
            w_r = w.rearrange("(kt p) c -> p kt c", p=128)
            for wq in range(4):     # split so the first matmuls start sooner
                nc.sync.dma_start(out=w_sb[:, 4 * wq:4 * (wq + 1), :],
                                  in_=w_r[:, 4 * wq:4 * (wq + 1), :])
            wo_sb = p_const.tile([128, HPC, D], bf16)               # 8KB/p
            nc.sync.dma_start(out=wo_sb, in_=wo.rearrange("(h p) n -> p h n", p=128))
            cos_sb = p_const.tile([128, S], f32)
            nc.sync.dma_start(out=cos_sb, in_=cos)
            sin_sb = p_const.tile([128, S], f32)
            nc.sync.dma_start(out=sin_sb, in_=sin)
            mask_sb = p_const.tile([128, QC + 384], bf16)
            nc.sync.dma_start(out=mask_sb, in_=msk)
            ones_sb = p_const.tile([128, 128], bf16)
            nc.vector.memset(ones_sb, 1.0)
            ident = p_const.tile([128, 128], f32)
            masks.make_identity(nc, ident)

            for b in range(B):
                # ---- stage A: qkvT = W_qkvT.T @ xT[b], rope, v transpose ---
                qk_sb = p_batch.tile([128, 4, S], f32r, tag="qk")   # q0 q1 k0 k1
                v_sb = p_batch.tile([128, HPC, NKT, DH], bf16, tag="v")

                for sc in range(N_SC):
                    ss = slice(sc * SC, (sc + 1) * SC)
                    x_t = p_x.tile([128, NKT, SC], f32r, tag="x")
                    nc.sync.dma_start(
                        out=x_t,
                        in_=xT[b, :, ss].rearrange("(kt p) s -> p kt s", p=128),
                    )
                    vt_c = p_vt.tile([128, HPC, SC], f32, tag="vt")
                    for m in range(3 * HPC):
                        acc = ps_main.tile([128, 512], f32, tag="psA")
                        for kt in range(NKT):
                            nc.tensor.matmul(
                                acc[:, :SC],
                                w_sb[:, kt, m * 128:(m + 1) * 128],
                                x_t[:, kt, :],
                                start=(kt == 0), stop=(kt == NKT - 1),
                            )
                        if m < 2 * HPC:  # q or k head: rope
                            tmp = p_tmp.tile([128, SC], f32, tag="rope")
                            nc.vector.tensor_mul(
                                tmp[0:64, :], acc[64:128, :SC], sin_sb[0:64, ss])
                            nc.vector.tensor_mul(
                                tmp[64:128, :], acc[0:64, :SC], sin_sb[64:128, ss])
                            t2 = p_tmp.tile([128, SC], f32, tag="rope")
                            nc.vector.tensor_mul(t2, acc[:, :SC], cos_sb[:, ss])
                            nc.vector.tensor_add(qk_sb[:, m, ss], t2, tmp)
                        else:        # v head: stash f32 vT chunk
                            h = m - 2 * HPC
                            nc.scalar.copy(out=vt_c[:, h, :], in_=acc[:, :SC])
                    # transpose this chunk's v tiles to natural [k, dh] layout
                    for h in range(HPC):
                        for j in range(SC // 128):
                            kt = (sc * SC) // 128 + j
                            pt = ps_main.tile([128, 512], f32, tag="psA")
                            nc.tensor.transpose(
                                pt[:, :128], vt_c[:, h, j * 128:(j + 1) * 128], ident)
                            nc.vector.tensor_copy(
                                out=v_sb[:, h, kt, :], in_=pt[:, :128])

                # ---- attention + interleaved out-projection, per q-chunk ---
                for qc in reversed(range(N_QC)):
                    qs = slice(qc * QC, (qc + 1) * QC)
                    nkt = (qc + 1) * (QC // 128)
                    zq = p_batch.tile([128, HPC, QC], bf16, tag=f"zT{qc}",
                                      name=f"zq{qc}")
                    for h in range(HPC):
                        qT = qk_sb[:, h, :]
                        kT = qk_sb[:, HPC + h, :]
                        den = ps_den.tile([128, 512], f32, tag="den")
                        zps = ps_z.tile([128, 512], f32, tag="z")
                        prs = []
                        offs = []
                        for kt in range(nkt):
                            j = kt - qc * (QC // 128)
                            off = 128 * j if j > 0 else 0   # causal col offset
                            ncols = QC - off
                            # fp32r needs moving dim >= 256; N=128 costs as
                            # much as N=512, so don't slice the j=3 scores
                            s_off = off if ncols >= 256 else 0
                            ps = ps_main.tile([128, 512], f32, tag="psA")
                            nc.tensor.matmul(
                                ps[:, s_off:], kT[:, kt * 128:(kt + 1) * 128],
                                qT[:, qc * QC + s_off:(qc + 1) * QC],
                                start=True, stop=True)
                            pr = p_probs.tile([128, QC], bf16, tag="pr")
                            nc.scalar.activation(out=pr[:, off:], in_=ps[:, off:],
                                                 func=Exp, scale=SM_SCALE)
                            if j >= 0:
                                nc.vector.tensor_mul(
                                    pr[:, off:], pr[:, off:],
                                    mask_sb[:, 384:384 + ncols])
                            prs.append(pr)
                            offs.append(off)
                            nc.tensor.matmul(zps[:, off:], v_sb[:, h, kt, :],
                                             pr[:, off:],
                                             start=(kt == 0), stop=(kt == nkt - 1))
                            if kt % 4 == 3:  # flush den group (ones LDW 1x/group)
                                for kk in range(kt - 3, kt + 1):
                                    o = offs[kk]
                                    nc.tensor.matmul(
                                        den[:, o:], ones_sb, prs[kk][:, o:],
                                        start=(kk == 0), stop=(kk == nkt - 1))
                        rec = p_rec.tile([128, 512], f32, tag="rec")
                        nc.vector.reciprocal_approx_fast(out=rec, in_=den)
                        nc.vector.tensor_mul(zq[:, h, :], zps, rec)

                    # out-proj rows covered by this q-chunk
                    for mi in range(QC // 128):
                        mt = qc * (QC // 128) + mi
                        ms = slice(mt * 128, (mt + 1) * 128)
                        mloc = slice(mi * 128, (mi + 1) * 128)
                        for pair in range(2):
                            po = [ps_o.tile([128, 512], f32, tag="po",
                                            name=f"po{i}") for i in range(2)]
                            for h in range(HPC):
                                for i in range(2):
                                    ncx = 2 * pair + i
                                    nc.tensor.matmul(
                                        po[i],
                                        zq[:, h, mloc],
                                        wo_sb[:, h, ncx * 512:(ncx + 1) * 512],
                                        start=(h == 0), stop=(h == HPC - 1),
                                    )
                            for i in range(2):
                                ncx = 2 * pair + i
                                ost = p_ost.tile([128, 512], f32, tag="ost",
                                                 name=f"ost{i}")
                                if i == 0:
                                    nc.scalar.copy(out=ost, in_=po[i])
                                else:
                                    nc.vector.tensor_copy(out=ost, in_=po[i])
                                nc.sync.dma_start(
                                    out=out[b, ms, ncx * 512:(ncx + 1) * 512],
                                    in_=ost)
    nc.finalize()
    return nc


def _rope_tables():
    inv_freq = 1.0 / (ROPE_BASE ** (np.arange(0, DH, 2, dtype=np.float64) / DH))
    t = np.arange(S, dtype=np.float64)
    freqs = np.outer(t, inv_freq)                       # [S, 64]
    emb = np.concatenate([freqs, freqs], axis=1)        # [S, 128]
    cosT = np.cos(emb).T.astype(np.float32).copy()      # [128, S]
    sinT = np.sin(emb).T.astype(np.float32)
    sin_mod = sinT.copy()
    sin_mod[:64] = -sin_mod[:64]                        # rotate-half sign fold
    return cosT, np.ascontiguousarray(sin_mod)


def _mask_tiles():
    r = np.arange(128)[:, None]
    cc = np.arange(QC + 384)[None, :]
    m = (r <= cc - 384)                                 # shifted causal window
    return m.astype(ml_dtypes.bfloat16)


def kernel(x, W_qkv, W_o):
    global LAST_RESULTS
    if "nc" not in _NC_CACHE:
        _NC_CACHE["nc"] = build_nc()
    nc = _NC_CACHE["nc"]

    x = np.asarray(x, dtype=np.float32)
    W_qkv = np.asarray(W_qkv, dtype=np.float32)
    W_o = np.asarray(W_o, dtype=np.float32)

    xT = np.ascontiguousarray(x.transpose(0, 2, 1))     # [B, D, S]
    WT = np.ascontiguousarray(W_qkv.T)                  # [D, 3D]
    WoT = np.ascontiguousarray(W_o.T)                   # [D, D] rows = z dims
    cosT, sinT = _rope_tables()
    mtiles = _mask_tiles()

    in_maps = []
    for c in range(N_CORES):
        h0 = c * HPC
        cols = []
        for part in range(3):                            # q, k, v column groups
            base = part * D + h0 * DH
            cols.append(WT[:, base: base + HPC * DH])
        wqkvT_local = np.ascontiguousarray(np.concatenate(cols, axis=1))
        woT_local = np.ascontiguousarray(
            WoT[h0 * DH:(h0 + HPC) * DH, :]).astype(ml_dtypes.bfloat16)
        in_maps.append({
            "xT": xT,
            "wqkvT": wqkvT_local,
            "woT": woT_local,
            "cosT": cosT,
            "sinT": sinT,
            "masks": mtiles,
        })

    trace = bool(int(os.environ.get("ATTN_TRACE", "0")))
    res = run_bass_kernel_spmd(nc, in_maps, list(range(N_CORES)), trace=trace)
    LAST_RESULTS = res
    partials = np.stack([res.results[c]["out"] for c in range(N_CORES)])
    return partials.sum(axis=0, dtype=np.float64).astype(np.float32)


# revision 13
# speedup vs baseline: 1.1024x; 1.1024x over previous
"""Multi-head causal attention w/ RoPE on 8 Trainium2 NeuronCores.

Tensor-parallel over heads: each core owns 2 of 16 heads (both batches),
computes qkv projection / RoPE / attention / output projection for its
heads, and returns a partial [2, 2048, 2048] output (its heads' slice of
the residual). The host sums the 8 partials (the K-split of the output
projection), which is the unshard for this sharding.

Precision: fp32r (TF32-like, full PE rate) for qkv/scores matmuls; bf16
for probs @ v and the output projection. Softmax skips max-subtraction
(scores are bounded ~25 here; exp stays far from fp32 overflow).
"""
import os
import sys

for _p in ("/root/.axon_site", "/root/.axon_site/_ro/trn_rl_repo",
           "/root/.axon_site/_ro/pypackages", "/opt/trn_rl_repo"):
    if os.path.isdir(_p) and _p not in sys.path:
        sys.path.append(_p)

import numpy as np
import ml_dtypes

import concourse.bacc as bacc
import concourse.tile as tile
from concourse import mybir, masks
from concourse.bass_utils import run_bass_kernel_spmd

N_CORES = 8
B = 2
S = 2048
D = 2048
H = 16
DH = 128
HPC = H // N_CORES          # heads per core = 2
ROPE_BASE = 10000.0
SM_SCALE = DH ** -0.5

SC = 512                    # stage-A s-chunk: one N=512 matmul per stationary
QC = 512                    # attention q-chunk
NKT = S // 128              # 16 k-tiles per sequence
N_SC = S // SC
N_QC = S // QC

f32 = mybir.dt.float32
f32r = mybir.dt.float32r
bf16 = mybir.dt.bfloat16
Exp = mybir.ActivationFunctionType.Exp

LAST_RESULTS = None         # BassKernelResults of the most recent run
_NC_CACHE = {}


def build_nc():
    nc = bacc.Bacc(trn_type="TRN2", debug=False)

    xT = nc.dram_tensor("xT", [B, D, S], f32r, kind="ExternalInput").ap()
    w = nc.dram_tensor("wqkvT", [D, 3 * HPC * DH], f32r, kind="ExternalInput").ap()
    wo = nc.dram_tensor("woT", [HPC * DH, D], bf16, kind="ExternalInput").ap()
    cos = nc.dram_tensor("cosT", [DH, S], f32, kind="ExternalInput").ap()
    sin = nc.dram_tensor("sinT", [DH, S], f32, kind="ExternalInput").ap()
    msk = nc.dram_tensor("masks", [DH, QC + 384], bf16, kind="ExternalInput").ap()
    out = nc.dram_tensor("out", [B, S, D], f32, kind="ExternalOutput").ap()

    with tile.TileContext(nc) as tc:
        with (
            tc.tile_pool(name="const", bufs=1) as p_const,
            tc.tile_pool(name="batch", bufs=1) as p_batch,
            tc.tile_pool(name="xch", bufs=2) as p_x,
            tc.tile_pool(name="vtc", bufs=1) as p_vt,
            tc.tile_pool(name="tmp", bufs=2) as p_tmp,
            tc.tile_pool(name="probs", bufs=6) as p_probs,
            tc.tile_pool(name="rec", bufs=1) as p_rec,
            tc.tile_pool(name="ostage", bufs=2) as p_ost,
            tc.tile_pool(name="psMain", bufs=3, space="PSUM") as ps_main,
            tc.tile_pool(name="psDen", bufs=1, space="PSUM") as ps_den,
            tc.tile_pool(name="psZ", bufs=2, space="PSUM") as ps_z,
            tc.tile_pool(name="psO", bufs=2, space="PSUM") as ps_o,
        ):
            # ---- constants -------------------------------------------------
            w_sb = p_const.tile([128, NKT, 3 * HPC * DH], f32r)     # 48KB/p
            w_r = w.rearrange("(kt p) c -> p kt c", p=128)
            for wq in range(4):     # split so the first matmuls start sooner
                nc.sync.dma_start(out=w_sb[:, 4 * wq:4 * (wq + 1), :],
                                  in_=w_r[:, 4 * wq:4 * (wq + 1), :])
            wo_sb = p_const.tile([128, HPC, D], bf16)               # 8KB/p
            nc.sync.dma_start(out=wo_sb, in_=wo.rearrange("(h p) n -> p h n", p=128))
            cos_sb = p_const.tile([128, S], f32)
            nc.sync.dma_start(out=cos_sb, in_=cos)
            sin_sb = p_const.tile([128, S], f32)
            nc.sync.dma_start(out=sin_sb, in_=sin)
            mask_sb = p_const.tile([128, QC + 384], bf16)
            nc.sync.dma_start(out=mask_sb, in_=msk)
            ones_sb = p_const.tile([128, 128], bf16)
            nc.vector.memset(ones_sb, 1.0)
            ident = p_const.tile([128, 128], f32)
            masks.make_identity(nc, ident)

            for b in range(B):
                # ---- stage A: qkvT = W_qkvT.T @ xT[b], rope, v transpose ---
                qk_sb = p_batch.tile([128, 4, S], f32r, tag="qk")   # q0 q1 k0 k1
                v_sb = p_batch.tile([128, HPC, NKT, DH], bf16, tag="v")

                for sc in range(N_SC):
                    ss = slice(sc * SC, (sc + 1) * SC)
                    x_t = p_x.tile([128, NKT, SC], f32r, tag="x")
                    nc.sync.dma_start(
                        out=x_t,
                        in_=xT[b, :, ss].rearrange("(kt p) s -> p kt s", p=128),
                    )
                    vt_c = p_vt.tile([128, HPC, SC], f32, tag="vt")
                    for m in range(3 * HPC):
                        acc = ps_main.tile([128, 512], f32, tag="psA")
                        for kt in range(NKT):
                            nc.tensor.matmul(
                                acc[:, :SC],
                                w_sb[:, kt, m * 128:(m + 1) * 128],
                                x_t[:, kt, :],
                                start=(kt == 0), stop=(kt == NKT - 1),
                            )
                        if m < 2 * HPC:  # q or k head: rope
                            tmp = p_tmp.tile([128, SC], f32, tag="rope")
                            nc.vector.tensor_mul(
                                tmp[0:64, :], acc[64:128, :SC], sin_sb[0:64, ss])
                            nc.vector.tensor_mul(
                                tmp[64:128, :], acc[0:64, :SC], sin_sb[64:128, ss])
                            t2 = p_tmp.tile([128, SC], f32, tag="rope")
                            nc.vector.tensor_mul(t2, acc[:, :SC], cos_sb[:, ss])
                            nc.vector.tensor_add(qk_sb[:, m, ss], t2, tmp)
                        else:        # v head: stash f32 vT chunk
                            h = m - 2 * HPC
                            nc.scalar.copy(out=vt_c[:, h, :], in_=acc[:, :SC])
                    # transpose this chunk's v tiles to natural [k, dh] layout
                    for h in range(HPC):
                        for j in range(SC // 128):
                            kt = (sc * SC) // 128 + j
                            pt = ps_main.tile([128, 512], f32, tag="psA")
                            nc.tensor.transpose(
                                pt[:, :128], vt_c[:, h, j * 128:(j + 1) * 128], ident)
                            nc.vector.tensor_copy(
                                out=v_sb[:, h, kt, :], in_=pt[:, :128])

                # ---- attention + interleaved out-projection, per q-chunk ---
                for qc in range(N_QC):
                    qs = slice(qc * QC, (qc + 1) * QC)
                    nkt = (qc + 1) * (QC // 128)
                    zq = p_batch.tile([128, HPC, QC], bf16, tag=f"zT{qc}",
                                      name=f"zq{qc}")
                    for h in range(HPC):
                        qT = qk_sb[:, h, :]
                        kT = qk_sb[:, HPC + h, :]
                        den = ps_den.tile([128, 512], f32, tag="den")
                        zps = ps_z.tile([128, 512], f32, tag="z")
                        prs = []
                        offs = []
                        for kt in range(nkt):
                            j = kt - qc * (QC // 128)
                            off = 128 * j if j > 0 else 0   # causal col offset
                            ncols = QC - off
                            # fp32r needs moving dim >= 256; N=128 costs as
                            # much as N=512, so don't slice the j=3 scores
                            s_off = off if ncols >= 256 else 0
                            ps = ps_main.tile([128, 512], f32, tag="psA")
                            nc.tensor.matmul(
                                ps[:, s_off:], kT[:, kt * 128:(kt + 1) * 128],
                                qT[:, qc * QC + s_off:(qc + 1) * QC],
                                start=True, stop=True)
                            pr = p_probs.tile([128, QC], bf16, tag="pr")
                            nc.scalar.activation(out=pr[:, off:], in_=ps[:, off:],
                                                 func=Exp, scale=SM_SCALE)
                            if j >= 0:
                                nc.vector.tensor_mul(
                                    pr[:, off:], pr[:, off:],
                                    mask_sb[:, 384:384 + ncols])
                            prs.append(pr)
                            offs.append(off)
                            nc.tensor.matmul(zps[:, off:], v_sb[:, h, kt, :],
                                             pr[:, off:],
                                             start=(kt == 0), stop=(kt == nkt - 1))
                            if kt % 4 == 3:  # flush den group (ones LDW 1x/group)
                                for kk in range(kt - 3, kt + 1):
                                    o = offs[kk]
                                    nc.tensor.matmul(
                                        den[:, o:], ones_sb, prs[kk][:, o:],
                                        start=(kk == 0), stop=(kk == nkt - 1))
                        rec = p_rec.tile([128, 512], f32, tag="rec")
                        nc.vector.reciprocal_approx_fast(out=rec, in_=den)
                        nc.vector.tensor_mul(zq[:, h, :], zps, rec)

                    # out-proj rows covered by this q-chunk
                    for mi in range(QC // 128):
                        mt = qc * (QC // 128) + mi
                        ms = slice(mt * 128, (mt + 1) * 128)
                        mloc = slice(mi * 128, (mi + 1) * 128)
                        for pair in range(2):
                            po = [ps_o.tile([128, 512], f32, tag="po",
                                            name=f"po{i}") for i in range(2)]
                            for h in range(HPC):
                                for i in range(2):
                                    ncx = 2 * pair + i
                                    nc.tensor.matmul(
                                        po[i],
                                        zq[:, h, mloc],
                                        wo_sb[:, h, ncx * 512:(ncx + 1) * 512],
                                        start=(h == 0), stop=(h == HPC - 1),
                                    )
                            for i in range(2):
                                ncx = 2 * pair + i
                                ost = p_ost.tile([128, 512], f32, tag="ost",
                                                 name=f"ost{i}")
                                if i == 0:
                                    nc.scalar.copy(out=ost, in_=po[i])
                                else:
                                    nc.vector.tensor_copy(out=ost, in_=po[i])
                                nc.sync.dma_start(
                                    out=out[b, ms, ncx * 512:(ncx + 1) * 512],
                                    in_=ost)
    nc.finalize()
    return nc


def _rope_tables():
    inv_freq = 1.0 / (ROPE_BASE ** (np.arange(0, DH, 2, dtype=np.float64) / DH))
    t = np.arange(S, dtype=np.float64)
    freqs = np.outer(t, inv_freq)                       # [S, 64]
    emb = np.concatenate([freqs, freqs], axis=1)        # [S, 128]
    cosT = np.cos(emb).T.astype(np.float32).copy()      # [128, S]
    sinT = np.sin(emb).T.astype(np.float32)
    sin_mod = sinT.copy()
    sin_mod[:64] = -sin_mod[:64]                        # rotate-half sign fold
    return cosT, np.ascontiguousarray(sin_mod)


def _mask_tiles():
    r = np.arange(128)[:, None]
    cc = np.arange(QC + 384)[None, :]
    m = (r <= cc - 384)                                 # shifted causal window
    return m.astype(ml_dtypes.bfloat16)


def kernel(x, W_qkv, W_o):
    global LAST_RESULTS
    if "nc" not in _NC_CACHE:
        _NC_CACHE["nc"] = build_nc()
    nc = _NC_CACHE["nc"]

    x = np.asarray(x, dtype=np.float32)
    W_qkv = np.asarray(W_qkv, dtype=np.float32)
    W_o = np.asarray(W_o, dtype=np.float32)

    xT = np.ascontiguousarray(x.transpose(0, 2, 1))     # [B, D, S]
    WT = np.ascontiguousarray(W_qkv.T)                  # [D, 3D]
    WoT = np.ascontiguousarray(W_o.T)                   # [D, D] rows = z dims
    cosT, sinT = _rope_tables()
    mtiles = _mask_tiles()

    in_maps = []
    for c in range(N_CORES):
        h0 = c * HPC
        cols = []
        for part in range(3):                            # q, k, v column groups
            base = part * D + h0 * DH
            cols.append(WT[:, base: base + HPC * DH])
        wqkvT_local = np.ascontiguousarray(np.concatenate(cols, axis=1))
        woT_local = np.ascontiguousarray(
            WoT[h0 * DH:(h0 + HPC) * DH, :]).astype(ml_dtypes.bfloat16)
        in_maps.append({
            "xT": xT,
            "wqkvT": wqkvT_local,
            "woT": woT_local,
            "cosT": cosT,
            "sinT": sinT,
            "masks": mtiles,
        })

    trace = bool(int(os.environ.get("ATTN_TRACE", "0")))
    res = run_bass_kernel_spmd(nc, in_maps, list(range(N_CORES)), trace=trace)
    LAST_RESULTS = res
    partials = np.stack([res.results[c]["out"] for c in range(N_CORES)])
    return partials.sum(axis=0, dtype=np.float64).astype(np.float32)


# revision 14
# speedup vs baseline: 1.1421x; 1.0361x over previous
"""Multi-head causal attention w/ RoPE on 8 Trainium2 NeuronCores.

Tensor-parallel over heads: each core owns 2 of 16 heads (both batches),
computes qkv projection / RoPE / attention / output projection for its
heads, and returns a partial [2, 2048, 2048] output (its heads' slice of
the residual). The host sums the 8 partials (the K-split of the output
projection), which is the unshard for this sharding.

Precision: fp32r (TF32-like, full PE rate) for qkv/scores matmuls; bf16
for probs @ v and the output projection. Softmax skips max-subtraction
(scores are bounded ~25 here; exp stays far from fp32 overflow).
"""
import os
import sys

for _p in ("/root/.axon_site", "/root/.axon_site/_ro/trn_rl_repo",
           "/root/.axon_site/_ro/pypackages", "/opt/trn_rl_repo"):
    if os.path.isdir(_p) and _p not in sys.path:
        sys.path.append(_p)

import numpy as np
import ml_dtypes

import concourse.bacc as bacc
import concourse.tile as tile
from concourse import mybir, masks
from concourse.bass_utils import run_bass_kernel_spmd

N_CORES = 8
B = 2
S = 2048
D = 2048
H = 16
DH = 128
HPC = H // N_CORES          # heads per core = 2
ROPE_BASE = 10000.0
SM_SCALE = DH ** -0.5

SC = 512                    # stage-A s-chunk: one N=512 matmul per stationary
QC = 512                    # attention q-chunk
NKT = S // 128              # 16 k-tiles per sequence
N_SC = S // SC
N_QC = S // QC

f32 = mybir.dt.float32
f32r = mybir.dt.float32r
bf16 = mybir.dt.bfloat16
Exp = mybir.ActivationFunctionType.Exp

LAST_RESULTS = None         # BassKernelResults of the most recent run
_NC_CACHE = {}


def build_nc():
    nc = bacc.Bacc(trn_type="TRN2", debug=False)

    xT = nc.dram_tensor("xT", [B, D, S], f32r, kind="ExternalInput").ap()
    w = nc.dram_tensor("wqkvT", [D, 3 * HPC * DH], f32r, kind="ExternalInput").ap()
    wo = nc.dram_tensor("woT", [HPC * DH, D], bf16, kind="ExternalInput").ap()
    cos = nc.dram_tensor("cosT", [DH, S], f32, kind="ExternalInput").ap()
    sin = nc.dram_tensor("sinT", [DH, S], f32, kind="ExternalInput").ap()
    msk = nc.dram_tensor("masks", [DH, QC + 384], bf16, kind="ExternalInput").ap()
    out = nc.dram_tensor("out", [B, S, D], f32, kind="ExternalOutput").ap()

    with tile.TileContext(nc) as tc:
        with (
            tc.tile_pool(name="const", bufs=1) as p_const,
            tc.tile_pool(name="batch", bufs=1) as p_batch,
            tc.tile_pool(name="xch", bufs=2) as p_x,
            tc.tile_pool(name="vtc", bufs=1) as p_vt,
            tc.tile_pool(name="tmp", bufs=2) as p_tmp,
            tc.tile_pool(name="probs", bufs=6) as p_probs,
            tc.tile_pool(name="rec", bufs=1) as p_rec,
            tc.tile_pool(name="ostage", bufs=2) as p_ost,
            tc.tile_pool(name="psMain", bufs=3, space="PSUM") as ps_main,
            tc.tile_pool(name="psDen", bufs=1, space="PSUM") as ps_den,
            tc.tile_pool(name="psZ", bufs=2, space="PSUM") as ps_z,
            tc.tile_pool(name="psO", bufs=2, space="PSUM") as ps_o,
        ):
            # ---- constants -------------------------------------------------
            # Pre-issue the first x chunk so the first matmuls don't wait
            # for every constant load (startup is HBM-bound).
            x_first = p_x.tile([128, NKT, SC], f32r, tag="x")
            nc.sync.dma_start(
                out=x_first,
                in_=xT[0, :, 0:SC].rearrange("(kt p) s -> p kt s", p=128))
            w_r = w.rearrange("(kt p) c -> p kt c", p=128)
            w_lo = p_const.tile([128, NKT // 2, 3 * HPC * DH], f32r)  # 24KB/p
            nc.sync.dma_start(out=w_lo, in_=w_r[:, :NKT // 2, :])
            w_hi = p_const.tile([128, NKT // 2, 3 * HPC * DH], f32r)  # 24KB/p
            nc.sync.dma_start(out=w_hi, in_=w_r[:, NKT // 2:, :])
            wo_sb = p_const.tile([128, HPC, D], bf16)               # 8KB/p
            nc.sync.dma_start(out=wo_sb, in_=wo.rearrange("(h p) n -> p h n", p=128))
            cos_sb = p_const.tile([128, S], f32)
            nc.sync.dma_start(out=cos_sb, in_=cos)
            sin_sb = p_const.tile([128, S], f32)
            nc.sync.dma_start(out=sin_sb, in_=sin)
            mask_sb = p_const.tile([128, QC + 384], bf16)
            nc.sync.dma_start(out=mask_sb, in_=msk)
            ones_sb = p_const.tile([128, 128], bf16)
            nc.vector.memset(ones_sb, 1.0)
            ident = p_const.tile([128, 128], f32)
            masks.make_identity(nc, ident)

            for b in range(B):
                # ---- stage A: qkvT = W_qkvT.T @ xT[b], rope, v transpose ---
                qk_sb = p_batch.tile([128, 4, S], f32r, tag="qk")   # q0 q1 k0 k1
                v_sb = p_batch.tile([128, HPC, NKT, DH], bf16, tag="v")

                for sc in range(N_SC):
                    ss = slice(sc * SC, (sc + 1) * SC)
                    if b == 0 and sc == 0:
                        x_t = x_first
                    else:
                        x_t = p_x.tile([128, NKT, SC], f32r, tag="x")
                        nc.sync.dma_start(
                            out=x_t,
                            in_=xT[b, :, ss].rearrange("(kt p) s -> p kt s", p=128),
                        )
                    vt_c = p_vt.tile([128, HPC, SC], f32, tag="vt")
                    for m in range(3 * HPC):
                        acc = ps_main.tile([128, 512], f32, tag="psA")
                        for kt in range(NKT):
                            wt = w_lo if kt < NKT // 2 else w_hi
                            nc.tensor.matmul(
                                acc[:, :SC],
                                wt[:, kt % (NKT // 2), m * 128:(m + 1) * 128],
                                x_t[:, kt, :],
                                start=(kt == 0), stop=(kt == NKT - 1),
                            )
                        if m < 2 * HPC:  # q or k head: rope
                            tmp = p_tmp.tile([128, SC], f32, tag="rope")
                            nc.vector.tensor_mul(
                                tmp[0:64, :], acc[64:128, :SC], sin_sb[0:64, ss])
                            nc.vector.tensor_mul(
                                tmp[64:128, :], acc[0:64, :SC], sin_sb[64:128, ss])
                            t2 = p_tmp.tile([128, SC], f32, tag="rope")
                            nc.vector.tensor_mul(t2, acc[:, :SC], cos_sb[:, ss])
                            nc.vector.tensor_add(qk_sb[:, m, ss], t2, tmp)
                        else:        # v head: stash f32 vT chunk
                            h = m - 2 * HPC
                            nc.scalar.copy(out=vt_c[:, h, :], in_=acc[:, :SC])
                    # transpose this chunk's v tiles to natural [k, dh] layout
                    for h in range(HPC):
                        for j in range(SC // 128):
                            kt = (sc * SC) // 128 + j
                            pt = ps_main.tile([128, 512], f32, tag="psA")
                            nc.tensor.transpose(
                                pt[:, :128], vt_c[:, h, j * 128:(j + 1) * 128], ident)
                            nc.vector.tensor_copy(
                                out=v_sb[:, h, kt, :], in_=pt[:, :128])

                # ---- attention + interleaved out-projection, per q-chunk ---
                for qc in range(N_QC):
                    qs = slice(qc * QC, (qc + 1) * QC)
                    nkt = (qc + 1) * (QC // 128)
                    zq = p_batch.tile([128, HPC, QC], bf16, tag=f"zT{qc}",
                                      name=f"zq{qc}")
                    for h in range(HPC):
                        qT = qk_sb[:, h, :]
                        kT = qk_sb[:, HPC + h, :]
                        den = ps_den.tile([128, 512], f32, tag="den")
                        zps = ps_z.tile([128, 512], f32, tag="z")
                        prs = []
                        offs = []
                        for kt in range(nkt):
                            j = kt - qc * (QC // 128)
                            off = 128 * j if j > 0 else 0   # causal col offset
                            ncols = QC - off
                            # fp32r needs moving dim >= 256; N=128 costs as
                            # much as N=512, so don't slice the j=3 scores
                            s_off = off if ncols >= 256 else 0
                            ps = ps_main.tile([128, 512], f32, tag="psA")
                            nc.tensor.matmul(
                                ps[:, s_off:], kT[:, kt * 128:(kt + 1) * 128],
                                qT[:, qc * QC + s_off:(qc + 1) * QC],
                                start=True, stop=True)
                            pr = p_probs.tile([128, QC], bf16, tag="pr")
                            nc.scalar.activation(out=pr[:, off:], in_=ps[:, off:],
                                                 func=Exp, scale=SM_SCALE)
                            if j >= 0:
                                nc.vector.tensor_mul(
                                    pr[:, off:], pr[:, off:],
                                    mask_sb[:, 384:384 + ncols])
                            prs.append(pr)
                            offs.append(off)
                            nc.tensor.matmul(zps[:, off:], v_sb[:, h, kt, :],
                                             pr[:, off:],
                                             start=(kt == 0), stop=(kt == nkt - 1))
                            if kt % 4 == 3:  # flush den group (ones LDW 1x/group)
                                for kk in range(kt - 3, kt + 1):
                                    o = offs[kk]
                                    nc.tensor.matmul(
                                        den[:, o:], ones_sb, prs[kk][:, o:],
                                        start=(kk == 0), stop=(kk == nkt - 1))
                        rec = p_rec.tile([128, 512], f32, tag="rec")
                        nc.vector.reciprocal_approx_fast(out=rec, in_=den)
                        nc.vector.tensor_mul(zq[:, h, :], zps, rec)

                    # out-proj rows covered by this q-chunk
                    for mi in range(QC // 128):
                        mt = qc * (QC // 128) + mi
                        ms = slice(mt * 128, (mt + 1) * 128)
                        mloc = slice(mi * 128, (mi + 1) * 128)
                        for pair in range(2):
                            po = [ps_o.tile([128, 512], f32, tag="po",
                                            name=f"po{i}") for i in range(2)]
                            for h in range(HPC):
                                for i in range(2):
                                    ncx = 2 * pair + i
                                    nc.tensor.matmul(
                                        po[i],
                                        zq[:, h, mloc],
                                        wo_sb[:, h, ncx * 512:(ncx + 1) * 512],
                                        start=(h == 0), stop=(h == HPC - 1),
                                    )
                            for i in range(2):
                                ncx = 2 * pair + i
                                ost = p_ost.tile([128, 512], f32, tag="ost",
                                                 name=f"ost{i}")
                                if i == 0:
                                    nc.scalar.copy(out=ost, in_=po[i])
                                else:
                                    nc.vector.tensor_copy(out=ost, in_=po[i])
                                nc.sync.dma_start(
                                    out=out[b, ms, ncx * 512:(ncx + 1) * 512],
                                    in_=ost)
    nc.finalize()
    return nc


def _rope_tables():
    inv_freq = 1.0 / (ROPE_BASE ** (np.arange(0, DH, 2, dtype=np.float64) / DH))
    t = np.arange(S, dtype=np.float64)
    freqs = np.outer(t, inv_freq)                       # [S, 64]
    emb = np.concatenate([freqs, freqs], axis=1)        # [S, 128]
    cosT = np.cos(emb).T.astype(np.float32).copy()      # [128, S]
    sinT = np.sin(emb).T.astype(np.float32)
    sin_mod = sinT.copy()
    sin_mod[:64] = -sin_mod[:64]                        # rotate-half sign fold
    return cosT, np.ascontiguousarray(sin_mod)


def _mask_tiles():
    r = np.arange(128)[:, None]
    cc = np.arange(QC + 384)[None, :]
    m = (r <= cc - 384)                                 # shifted causal window
    return m.astype(ml_dtypes.bfloat16)


def kernel(x, W_qkv, W_o):
    global LAST_RESULTS
    if "nc" not in _NC_CACHE:
        _NC_CACHE["nc"] = build_nc()
    nc = _NC_CACHE["nc"]

    x = np.asarray(x, dtype=np.float32)
    W_qkv = np.asarray(W_qkv, dtype=np.float32)
    W_o = np.asarray(W_o, dtype=np.float32)

    xT = np.ascontiguousarray(x.transpose(0, 2, 1))     # [B, D, S]
    WT = np.ascontiguousarray(W_qkv.T)                  # [D, 3D]
    WoT = np.ascontiguousarray(W_o.T)                   # [D, D] rows = z dims
    cosT, sinT = _rope_tables()
    mtiles = _mask_tiles()

    in_maps = []
    for c in range(N_CORES):
        h0 = c * HPC
        cols = []
        for part in range(3):                            # q, k, v column groups
            base = part * D + h0 * DH
            cols.append(WT[:, base: base + HPC * DH])
        wqkvT_local = np.ascontiguousarray(np.concatenate(cols, axis=1))
        woT_local = np.ascontiguousarray(
            WoT[h0 * DH:(h0 + HPC) * DH, :]).astype(ml_dtypes.bfloat16)
        in_maps.append({
            "xT": xT,
            "wqkvT": wqkvT_local,
            "woT": woT_local,
            "cosT": cosT,
            "sinT": sinT,
            "masks": mtiles,
        })

    trace = bool(int(os.environ.get("ATTN_TRACE", "0")))
    res = run_bass_kernel_spmd(nc, in_maps, list(range(N_CORES)), trace=trace)
    LAST_RESULTS = res
    partials = np.stack([res.results[c]["out"] for c in range(N_CORES)])
    return partials.sum(axis=0, dtype=np.float64).astype(np.float32)
